# revision 48
# baseline (speedup 1.0000x reference)
"""Trainium2 Bass kernel: GroupNorm + single-head self-attention block.

Algebraically folded formulation (exact, softmax shift/i-invariance):
    xn   = groupnorm(x) * gamma + beta          (fp8 storage)
    mk   = (8 Wq^T Wk) @ xn                     # one proj replaces q,k
    sT   = mk^T xn                              # [j, i] = 8 * q_i . k_j
    e    = exp(sT * scale/8 - 2)                # shift cancels in softmax
    col  = sum_j e[j, i]
    vv   = (8 out_w Wv) @ xn / 8                # out-proj folded into v
    res  = vv @ e                               # [c, i]
    out  = x + (res + outb) / col               # outb = out_b + out_w bv

All big matmuls run fp8e4(m3) with perf_mode=DoubleRow (paired k-tiles,
2x PE rate). Folded weights are pre-scaled by 8 on the host so their
entries sit in fp8's normal range; the 8's cancel in exp-scale and the
vv copy. Numpy-simulated end-to-end absmax error vs the fp32 reference
is ~0.073 (rel 1.4e-2), within the 2e-2 gate.

Sharding: data-parallel over batch, 32 batches / 8 cores = 4 per core.
"""

import json
import os

import numpy as np
import ml_dtypes

import concourse.bass as bass
import concourse.mybir as mybir
import concourse.tile as tile
from concourse.bass_utils import run_bass_kernel_spmd


def _spill_multiwaits(raw: bytes) -> bytes:
    """Walrus in this toolchain accepts only one sync-wait command per
    instruction descriptor. Spill extra on_wait entries onto single-wait
    EventSemaphore instructions inserted immediately before, on the same
    engine queue (the exact pattern Tile's own barriers use), which is
    semantically identical: the queue blocks at the same point either way.
    """
    j = json.loads(raw)
    n = 0
    for fn in j.get("functions", []):
        for blk in fn.get("blocks", []):
            out = []
            for inst in blk.get("instructions", []):
                si = inst.get("sync_info") or {}
                waits = si.get("on_wait") or []
                if len(waits) > 1 and inst.get("engine"):
                    for spilled in waits[:-1]:
                        n += 1
                        out.append({
                            "debug": inst.get("debug", 0),
                            "engine": inst["engine"],
                            "ins": [],
                            "name": f"{inst['name']}-sw{n}",
                            "opcode": "EventSemaphore",
                            "outs": [],
                            "sync_info": {"on_update": [], "on_wait": [spilled]},
                        })
                    si["on_wait"] = waits[-1:]
                out.append(inst)
            blk["instructions"] = out
    return json.dumps(j).encode()


_orig_to_json_bytes = bass.Bass.to_json_bytes


def _patched_to_json_bytes(self):
    return _spill_multiwaits(_orig_to_json_bytes(self))


bass.Bass.to_json_bytes = _patched_to_json_bytes

F32 = mybir.dt.float32
FP8 = mybir.dt.float8e4
MM_DT = mybir.dt.float32r
DR = mybir.MatmulPerfMode.DoubleRow

N_CORES = 8
B_TOTAL = 32
B_PER_CORE = B_TOTAL // N_CORES
C = 512
HW = 1024
GROUPS = 8
EPS = 1e-5
WS = 8.0                       # host pre-scale on folded weights
SCALE = float(C) ** -0.5
ESHIFT = -2.0                  # softmax shift: keeps exp under fp8 max

CT = C // 128   # 4 channel tiles
PT = HW // 128  # 8 pixel tiles
NB = HW // 512  # 2 free-dim blocks of 512
NP8 = ml_dtypes.float8_e4m3


def build_nc():
    nc = bass.Bass()

    x_d = nc.dram_tensor("x", [B_PER_CORE, C, HW], F32, kind="ExternalInput")
    wmT_d = nc.dram_tensor("wmT", [C, C], FP8, kind="ExternalInput")
    wovT_d = nc.dram_tensor("wovT", [C, C], FP8, kind="ExternalInput")
    outb_d = nc.dram_tensor("outb", [C], F32, kind="ExternalInput")
    gamma_d = nc.dram_tensor("gamma", [C], F32, kind="ExternalInput")
    beta_d = nc.dram_tensor("beta", [C], F32, kind="ExternalInput")
    sel_d = nc.dram_tensor("sel", [C, GROUPS], F32, kind="ExternalInput")
    selT_d = nc.dram_tensor("selT", [GROUPS, C], F32, kind="ExternalInput")
    out_d = nc.dram_tensor("out", [B_PER_CORE, C, HW], F32, kind="ExternalOutput")
    warmdump_d = nc.dram_tensor("warmdump", [128, 4], F32)

    with tile.TileContext(nc) as tc:
        with (
            tc.tile_pool(name="wpool", bufs=1) as wpool,
            tc.tile_pool(name="xpool", bufs=2) as xpool,
            tc.tile_pool(name="xnpool", bufs=2) as xnpool,
            tc.tile_pool(name="mkpool", bufs=2) as mkpool,
            tc.tile_pool(name="vtpool", bufs=2) as vtpool,
            tc.tile_pool(name="expool", bufs=2) as expool,
            tc.tile_pool(name="spool", bufs=2) as spool,
            tc.tile_pool(name="ftpool", bufs=2) as ftpool,
            tc.tile_pool(name="rpool", bufs=2) as rpool,
            tc.tile_pool(name="mmps", bufs=2, space=bass.MemorySpace.PSUM) as mmps,
            tc.tile_pool(name="colps", bufs=1, space=bass.MemorySpace.PSUM) as colpool,
            tc.tile_pool(name="stps", bufs=1, space=bass.MemorySpace.PSUM) as stps,
        ):
            xts = {}
            xns = {}
            stvs = {}

            def load_x(bb, fine=False):
                xt = xpool.tile([128, CT, HW], F32, tag="xt")
                xts[bb] = xt
                # per-c-tile chunks so bn_stats can start before the full
                # load; the prologue batch splits finer (per bn_stats chunk)
                # to start the very first stats op earlier
                for t in range(CT):
                    if fine:
                        for sg in range(2):
                            nc.sync.dma_start(
                                out=xt[:, t, sg * 512:(sg + 1) * 512],
                                in_=x_d[bb, t * 128:(t + 1) * 128,
                                        sg * 512:(sg + 1) * 512])
                    else:
                        nc.sync.dma_start(
                            out=xt[:, t],
                            in_=x_d[bb, t * 128:(t + 1) * 128, :])
                return xt

            load_x(0, fine=True)

            # ---- HAM warmup first: its DVE dependencies lead the DVE queue
            # so the PE starts warming ~5us earlier; keeps the clock gate at
            # 8/8 (2.4GHz) until the first real matmul is ready.
            ones_st = wpool.tile([128, 2, 128], F32)
            nc.vector.memset(ones_st, 1.0)
            ones_r = wpool.tile([128, 128], MM_DT)
            nc.vector.tensor_copy(ones_r, ones_st[:, 0])
            warm_st = wpool.tile([128, 512], F32)
            nc.vector.memset(warm_st, 0.0)
            warm_rhs = wpool.tile([128, 512], MM_DT)
            nc.vector.tensor_copy(warm_rhs, warm_st)
            # no consumer for warm_ps: a consumer on ANY engine queue blocks
            # that queue until warm-end (DVE: +12us on batch-0 stats; ACT:
            # stalls the prologue rstd/xn ops). The unread PSUM writes draw
            # only a compiler warning and the first real matmul group
            # overwrites the bank with start=True.
            warm_ps = stps.tile([128, 512], F32, tag="gps")
            for w in range(16):
                nc.tensor.matmul(warm_ps, lhsT=ones_r, rhs=warm_rhs,
                                 start=True, stop=True)

            # ---- tiny constants ----
            eps_sb = wpool.tile([128, 1], F32)
            nc.vector.memset(eps_sb, EPS)
            eshift_sb = wpool.tile([128, 1], F32)
            nc.vector.memset(eshift_sb, ESHIFT)
            ones_sb = wpool.tile([128, 2, 128], FP8)
            nc.vector.tensor_copy(ones_sb, ones_st)

            sel_st = wpool.tile([128, CT, GROUPS], F32)
            nc.sync.dma_start(out=sel_st, in_=sel_d.rearrange("(t p) g -> p t g", p=128))
            sel_sb = wpool.tile([128, CT, GROUPS], MM_DT)
            nc.vector.tensor_copy(sel_sb, sel_st)
            selT_st = wpool.tile([GROUPS, C], F32)
            nc.sync.dma_start(out=selT_st, in_=selT_d[:, :])
            selT_sb = wpool.tile([GROUPS, C], MM_DT)
            nc.vector.tensor_copy(selT_sb, selT_st)
            outb_sb = wpool.tile([128, CT], F32)
            nc.sync.dma_start(out=outb_sb, in_=outb_d.rearrange("(m p) -> p m", p=128))
            gamma_sb = wpool.tile([128, CT], F32)
            nc.sync.dma_start(out=gamma_sb, in_=gamma_d.rearrange("(m p) -> p m", p=128))
            beta_sb = wpool.tile([128, CT], F32)
            nc.sync.dma_start(out=beta_sb, in_=beta_d.rearrange("(m p) -> p m", p=128))

            # ---- folded fp8 weights ----
            wm_sb = wpool.tile([128, CT, C], FP8)
            wov_sb = wpool.tile([128, CT, C], FP8)
            wmT_r = wmT_d.rearrange("(t p) o -> p t o", p=128)
            wovT_r = wovT_d.rearrange("(t p) o -> p t o", p=128)
            for t in range(CT):
                nc.sync.dma_start(out=wm_sb[:, t], in_=wmT_r[:, t])
                nc.sync.dma_start(out=wov_sb[:, t], in_=wovT_r[:, t])

            def stats_front(bb):
                """bn stats chain for batch bb (DVE + tiny PE matmuls).
                Returns the group-stat PSUM tile; finish with stats_back."""
                xt = xts[bb]
                stats3 = spool.tile([128, CT, 4], F32, tag="stats3")
                stats3r = spool.tile([128, CT, 4], MM_DT, tag="stats3r")
                nc.vector.memset(stats3[:, :, 3:4], 0.0)
                gps = stps.tile([GROUPS, 4], F32, tag="gps")
                for t in range(CT):
                    st6 = spool.tile([128, 2, 6], F32, tag="st6")
                    for sg in range(2):
                        nc.vector.bn_stats(out=st6[:, sg], in_=xt[:, t, sg * 512:(sg + 1) * 512])
                    nc.vector.bn_aggr(out=stats3[:, t, 0:2], in_=st6)
                    nc.vector.tensor_mul(stats3[:, t, 2:3], stats3[:, t, 0:1], stats3[:, t, 0:1])
                    nc.vector.tensor_copy(stats3r[:, t], stats3[:, t])
                    nc.tensor.matmul(gps, lhsT=sel_sb[:, t], rhs=stats3r[:, t],
                                     start=(t == 0), stop=(t == CT - 1))
                return gps

            def stats_mid(bb, gps):
                """group var -> rstd (DVE small ops; sqrt emitted on ACT by
                caller-controlled ordering), then channel broadcast (PE)."""
                gsb = spool.tile([GROUPS, 4], F32, tag="gsb")
                nc.vector.tensor_copy(gsb, gps)
                gs = spool.tile([GROUPS, 4], F32, tag="gs")
                nc.vector.memset(gs, 0.0)
                tmp8 = spool.tile([GROUPS, 1], F32, tag="tmp8")
                nc.vector.tensor_mul(tmp8, gsb[:, 0:1], gsb[:, 0:1])
                nc.vector.tensor_add(gs[:, 1:2], gsb[:, 1:2], gsb[:, 2:3])
                nc.vector.tensor_sub(gs[:, 1:2], gs[:, 1:2], tmp8)
                # rstd = exp(-0.5*ln(var+eps)): stays inside ACT's resident
                # ln/exp table (a Sqrt here would force a table reload per
                # batch, ~1.5us each)
                nc.scalar.activation(gs[:, 1:2], gs[:, 1:2],
                                     mybir.ActivationFunctionType.Ln,
                                     bias=eps_sb[:GROUPS])
                nc.scalar.activation(gs[:, 1:2], gs[:, 1:2],
                                     mybir.ActivationFunctionType.Exp,
                                     scale=-0.5)
                nc.vector.tensor_copy(gs[:, 0:1], gsb[:, 0:1])
                gsr = spool.tile([GROUPS, 4], MM_DT, tag="gsr")
                nc.vector.tensor_copy(gsr, gs)
                csps = stps.tile([128, CT, 4], F32, tag="csps")
                for t in range(CT):
                    nc.tensor.matmul(csps[:, t], lhsT=selT_sb[:, t * 128:(t + 1) * 128],
                                     rhs=gsr, start=True, stop=True)
                # per-channel affine: xn = x * s + tt
                stv = spool.tile([128, CT, 2], F32, tag="stv")
                tmpc = spool.tile([128, CT, 1], F32, tag="tmpc")
                nc.vector.tensor_mul(stv[:, :, 0:1], csps[:, :, 1:2],
                                     gamma_sb.rearrange("p (t o) -> p t o", o=1))
                nc.vector.tensor_mul(tmpc, csps[:, :, 0:1], stv[:, :, 0:1])
                nc.vector.tensor_sub(stv[:, :, 1:2],
                                     beta_sb.rearrange("p (t o) -> p t o", o=1), tmpc)
                stvs[bb] = stv

            def xn_apply(bb, fast=False):
                """normalize+quantize x -> fp8 xn. Steady-state batches run
                serially on the (otherwise idle) gpsimd; the prologue batch
                fans out across three engines to cut pipeline-fill latency."""
                xt = xts[bb]
                stv = stvs.pop(bb)
                xn = xnpool.tile([128, CT, HW], FP8, tag="xn")
                xns[bb] = xn
                for t in range(CT):
                    if fast and t == 1:
                        nc.vector.tensor_scalar(
                            out=xn[:, t], in0=xt[:, t],
                            scalar1=stv[:, t, 0:1], scalar2=stv[:, t, 1:2],
                            op0=mybir.AluOpType.mult, op1=mybir.AluOpType.add)
                    elif fast and t == 3:
                        nc.scalar.activation(
                            xn[:, t], xt[:, t],
                            mybir.ActivationFunctionType.Identity,
                            bias=stv[:, t, 1:2], scale=stv[:, t, 0:1])
                    else:
                        nc.gpsimd.tensor_scalar(
                            out=xn[:, t], in0=xt[:, t],
                            scalar1=stv[:, t, 0:1], scalar2=stv[:, t, 1:2],
                            op0=mybir.AluOpType.mult, op1=mybir.AluOpType.add)
                return xn

            def proj(bb):
                """mk and vv projections (fp8 DoubleRow)."""
                xn = xns[bb]
                mk = mkpool.tile([128, CT, HW], FP8, tag="mk")
                for m in range(CT):
                    ps = mmps.tile([128, 1024], F32, tag="mm")
                    for n in range(NB):
                        for tp in range(2):
                            nc.tensor.matmul(
                                ps[:, n * 512:(n + 1) * 512],
                                lhsT=wm_sb[:, 2 * tp:2 * tp + 2, m * 128:(m + 1) * 128],
                                rhs=xn[:, 2 * tp:2 * tp + 2, n * 512:(n + 1) * 512],
                                start=(tp == 0), stop=(tp == 1), perf_mode=DR)
                    # all mk conversions on ACT: they run right after exp in
                    # the ACT queue, well before the next scores needs them.
                    # (A DVE split lands too late — behind the epilogue STTs
                    # in DVE's FIFO — stalling scores(bb+1) ~3us per batch.)
                    nc.scalar.activation(mk[:, m], ps,
                                         mybir.ActivationFunctionType.Copy)
                vT = vtpool.tile([128, PT, C], FP8, tag="vT")
                for pp in range(4):
                    ps = mmps.tile([128, 1024], F32, tag="mm")
                    for h in range(2):
                        p = 2 * pp + h
                        for tp in range(2):
                            nc.tensor.matmul(
                                ps[:, h * 512:(h + 1) * 512],
                                lhsT=xn[:, 2 * tp:2 * tp + 2, p * 128:(p + 1) * 128],
                                rhs=wov_sb[:, 2 * tp:2 * tp + 2, :],
                                start=(tp == 0), stop=(tp == 1), perf_mode=DR)
                    # undo the 8x host pre-scale of wov (gpsimd cannot read
                    # PSUM, so this conversion copy rides on ACT)
                    nc.scalar.activation(vT[:, 2 * pp:2 * pp + 2, :], ps,
                                         mybir.ActivationFunctionType.Copy,
                                         scale=1.0 / WS)
                return mk, vT

            def scores_exp(bb, mk, xn, interleave=None):
                """sT = mk^T xn then e = exp(sT*scale/8 - 2) -> fp8.
                interleave: optional list of (after_jm, fn) to slot extra
                engine work into the PE/ACT streams mid-phase."""
                expT = expool.tile([128, PT, HW], FP8, tag="expT")
                for jm in range(PT):
                    ps = mmps.tile([128, 1024], F32, tag="mm")
                    for n in range(NB):
                        for tp in range(2):
                            nc.tensor.matmul(
                                ps[:, n * 512:(n + 1) * 512],
                                lhsT=mk[:, 2 * tp:2 * tp + 2, jm * 128:(jm + 1) * 128],
                                rhs=xn[:, 2 * tp:2 * tp + 2, n * 512:(n + 1) * 512],
                                start=(tp == 0), stop=(tp == 1), perf_mode=DR)
                    nc.scalar.activation(expT[:, jm], ps,
                                         mybir.ActivationFunctionType.Exp,
                                         scale=SCALE / WS, bias=eshift_sb)
                    if interleave:
                        for after, fn in interleave:
                            if after == jm:
                                fn()
                return expT

            def colsum(bb, expT):
                colps = colpool.tile([128, HW], F32, tag="col")
                for n in range(NB):
                    for jp in range(4):
                        nc.tensor.matmul(
                            colps[:, n * 512:(n + 1) * 512],
                            lhsT=ones_sb,
                            rhs=expT[:, 2 * jp:2 * jp + 2, n * 512:(n + 1) * 512],
                            start=(jp == 0), stop=(jp == 3), perf_mode=DR)
                # 1/col as exp(-ln(col)): ACT table ops, ~3x cheaper than the
                # DVE RECIPROCAL microcode and off the DVE critical path
                lcol = rpool.tile([128, HW], F32, tag="lcol")
                nc.scalar.activation(lcol, colps,
                                     mybir.ActivationFunctionType.Ln)
                recip = rpool.tile([128, HW], F32, tag="recip")
                nc.scalar.activation(recip, lcol,
                                     mybir.ActivationFunctionType.Exp,
                                     scale=-1.0)
                return recip

            def attn_out(bb, vT, expT, recip):
                """res = vv @ e, then out = x + (res + outb)*recip, store."""
                xt = xts.pop(bb)
                for m in range(CT):
                    ps = mmps.tile([128, 1024], F32, tag="mm")
                    for n in range(NB):
                        for jp in range(4):
                            nc.tensor.matmul(
                                ps[:, n * 512:(n + 1) * 512],
                                lhsT=vT[:, 2 * jp:2 * jp + 2, m * 128:(m + 1) * 128],
                                rhs=expT[:, 2 * jp:2 * jp + 2, n * 512:(n + 1) * 512],
                                start=(jp == 0), stop=(jp == 3), perf_mode=DR)
                    ftmp = ftpool.tile([128, HW], F32, tag="ftmp")
                    nc.vector.scalar_tensor_tensor(
                        out=ftmp, in0=ps, scalar=outb_sb[:, m:m + 1], in1=recip,
                        op0=mybir.AluOpType.add, op1=mybir.AluOpType.mult)
                    # last batch: residual add on DVE (3.5x faster than the
                    # gpsimd Add) — it is the kernel's tail with nothing to
                    # overlap; other batches keep it on the idle gpsimd
                    if bb == B_PER_CORE - 1:
                        nc.vector.tensor_add(xt[:, m], ftmp, xt[:, m])
                    else:
                        nc.gpsimd.tensor_add(xt[:, m], ftmp, xt[:, m])
                    nc.sync.dma_start(
                        out=out_d[bb, m * 128:(m + 1) * 128, :],
                        in_=xt[:, m])

            # ---- software pipeline over batches ----
            # prologue: stats+apply for batch 0
            gps0 = stats_front(0)
            stats_mid(0, gps0)
            xn_apply(0, fast=True)
            for bb in range(B_PER_CORE):
                if bb + 1 < B_PER_CORE:
                    load_x(bb + 1)
                mk, vT = proj(bb)
                # next batch's stats chain is slotted into the scores phase:
                # DVE runs bn stats while PE is ACT(exp)-paced; the tiny
                # group matmuls ride between score chunks.
                pend = {}
                inter = None
                if bb + 1 < B_PER_CORE:
                    inter = [
                        (3, lambda b=bb + 1: pend.__setitem__('gps', stats_front(b))),
                        (5, lambda b=bb + 1: stats_mid(b, pend.pop('gps'))),
                        (6, lambda b=bb + 1: xn_apply(b)),
                    ]
                expT = scores_exp(bb, mk, xns.pop(bb), interleave=inter)
                recip = colsum(bb, expT)
                attn_out(bb, vT, expT, recip)
    return nc


_NC_CACHE = None


def kernel(x, norm_gamma, norm_beta, qkv_w, qkv_b, out_w, out_b):
    global _NC_CACHE
    if _NC_CACHE is None:
        _NC_CACHE = build_nc()
    nc = _NC_CACHE

    x = np.ascontiguousarray(np.asarray(x, np.float32).reshape(B_TOTAL, C, HW))
    qkv_w = np.asarray(qkv_w, np.float32)
    qkv_b = np.asarray(qkv_b, np.float32)
    out_w = np.asarray(out_w, np.float32)
    Wq, Wk, Wv = qkv_w[:C], qkv_w[C:2 * C], qkv_w[2 * C:]
    # folded weights, pre-scaled into fp8's normal range
    wmT = np.ascontiguousarray((WS * (Wq.T @ Wk)).T.astype(NP8))
    wovT = np.ascontiguousarray((WS * (out_w @ Wv)).T.astype(NP8))
    # v-bias contributes out_w @ bv to every pixel (softmax rows sum to 1)
    outb = np.ascontiguousarray(np.asarray(out_b, np.float32) + out_w @ qkv_b[2 * C:])
    # q/k biases shift scores by an i-only term (cancels in softmax) plus a
    # j-only term r_j = (Wk^T bq) . xn_j; zero for this model's inputs.
    rvec = Wk.T @ qkv_b[:C]
    assert np.allclose(rvec, 0.0) and np.allclose(qkv_b[:C], 0.0), \
        "nonzero q-bias not supported by folded kernel"
    gamma = np.ascontiguousarray(np.asarray(norm_gamma, np.float32))
    beta = np.ascontiguousarray(np.asarray(norm_beta, np.float32))
    cidx = np.arange(C)
    # each group = 64 channels; selector averages the 64 per-channel stats
    sel = np.ascontiguousarray((cidx[:, None] // (C // GROUPS) == np.arange(GROUPS)[None, :])
                               .astype(np.float32) / (C // GROUPS))
    selT = np.ascontiguousarray((np.arange(GROUPS)[:, None] == cidx[None, :] // (C // GROUPS))
                                .astype(np.float32))

    shared = {"wmT": wmT, "wovT": wovT, "outb": outb,
              "gamma": gamma, "beta": beta, "sel": sel, "selT": selT}
    in_maps = [{"x": x[c * B_PER_CORE:(c + 1) * B_PER_CORE], **shared}
               for c in range(N_CORES)]

    trace = bool(int(os.environ.get("KERNEL_TRACE", "0")))
    res = run_bass_kernel_spmd(nc, in_maps, list(range(N_CORES)), trace=trace)
    if trace and res.exec_time_ns is not None:
        print(f"HW exec time: {res.exec_time_ns} ns")
        print(f"(mean across cores: {res.mean_exec_time_ns} ns, "
              f"max core: {res.max_exec_time_core_id})")

    out = np.concatenate([res.results[c]["out"] for c in range(N_CORES)], axis=0)
    return out.reshape(B_TOTAL, C, 32, 32).astype(np.float32)


# revision 50
# speedup vs baseline: 1.0098x; 1.0098x over previous
"""Trainium2 Bass kernel: GroupNorm + single-head self-attention block.

Algebraically folded formulation (exact, softmax shift/i-invariance):
    xn   = groupnorm(x) * gamma + beta          (fp8 storage)
    mk   = (8 Wq^T Wk) @ xn                     # one proj replaces q,k
    sT   = mk^T xn                              # [j, i] = 8 * q_i . k_j
    e    = exp(sT * scale/8 - 2)                # shift cancels in softmax
    col  = sum_j e[j, i]
    vv   = (8 out_w Wv) @ xn / 8                # out-proj folded into v
    res  = vv @ e                               # [c, i]
    out  = x + (res + outb) / col               # outb = out_b + out_w bv

All big matmuls run fp8e4(m3) with perf_mode=DoubleRow (paired k-tiles,
2x PE rate). Folded weights are pre-scaled by 8 on the host so their
entries sit in fp8's normal range; the 8's cancel in exp-scale and the
vv copy. Numpy-simulated end-to-end absmax error vs the fp32 reference
is ~0.073 (rel 1.4e-2), within the 2e-2 gate.

Sharding: data-parallel over batch, 32 batches / 8 cores = 4 per core.
"""

import json
import os

import numpy as np
import ml_dtypes

import concourse.bass as bass
import concourse.mybir as mybir
import concourse.tile as tile
from concourse.bass_utils import run_bass_kernel_spmd


def _spill_multiwaits(raw: bytes) -> bytes:
    """Walrus in this toolchain accepts only one sync-wait command per
    instruction descriptor. Spill extra on_wait entries onto single-wait
    EventSemaphore instructions inserted immediately before, on the same
    engine queue (the exact pattern Tile's own barriers use), which is
    semantically identical: the queue blocks at the same point either way.
    """
    j = json.loads(raw)
    n = 0
    for fn in j.get("functions", []):
        for blk in fn.get("blocks", []):
            out = []
            for inst in blk.get("instructions", []):
                si = inst.get("sync_info") or {}
                waits = si.get("on_wait") or []
                if len(waits) > 1 and inst.get("engine"):
                    for spilled in waits[:-1]:
                        n += 1
                        out.append({
                            "debug": inst.get("debug", 0),
                            "engine": inst["engine"],
                            "ins": [],
                            "name": f"{inst['name']}-sw{n}",
                            "opcode": "EventSemaphore",
                            "outs": [],
                            "sync_info": {"on_update": [], "on_wait": [spilled]},
                        })
                    si["on_wait"] = waits[-1:]
                out.append(inst)
            blk["instructions"] = out
    return json.dumps(j).encode()


_orig_to_json_bytes = bass.Bass.to_json_bytes


def _patched_to_json_bytes(self):
    return _spill_multiwaits(_orig_to_json_bytes(self))


bass.Bass.to_json_bytes = _patched_to_json_bytes

F32 = mybir.dt.float32
FP8 = mybir.dt.float8e4
MM_DT = mybir.dt.float32r
DR = mybir.MatmulPerfMode.DoubleRow

N_CORES = 8
B_TOTAL = 32
B_PER_CORE = B_TOTAL // N_CORES
C = 512
HW = 1024
GROUPS = 8
EPS = 1e-5
WS = 8.0                       # host pre-scale on folded weights
SCALE = float(C) ** -0.5
ESHIFT = -2.0                  # softmax shift: keeps exp under fp8 max

CT = C // 128   # 4 channel tiles
PT = HW // 128  # 8 pixel tiles
NB = HW // 512  # 2 free-dim blocks of 512
NP8 = ml_dtypes.float8_e4m3


def build_nc():
    nc = bass.Bass()

    x_d = nc.dram_tensor("x", [B_PER_CORE, C, HW], F32, kind="ExternalInput")
    wmT_d = nc.dram_tensor("wmT", [C, C], FP8, kind="ExternalInput")
    wovT_d = nc.dram_tensor("wovT", [C, C], FP8, kind="ExternalInput")
    outb_d = nc.dram_tensor("outb", [C], F32, kind="ExternalInput")
    gamma_d = nc.dram_tensor("gamma", [C], F32, kind="ExternalInput")
    beta_d = nc.dram_tensor("beta", [C], F32, kind="ExternalInput")
    sel_d = nc.dram_tensor("sel", [C, GROUPS], F32, kind="ExternalInput")
    selT_d = nc.dram_tensor("selT", [GROUPS, C], F32, kind="ExternalInput")
    out_d = nc.dram_tensor("out", [B_PER_CORE, C, HW], F32, kind="ExternalOutput")
    warmdump_d = nc.dram_tensor("warmdump", [128, 4], F32)

    with tile.TileContext(nc) as tc:
        with (
            tc.tile_pool(name="wpool", bufs=1) as wpool,
            tc.tile_pool(name="xpool", bufs=2) as xpool,
            tc.tile_pool(name="xnpool", bufs=2) as xnpool,
            tc.tile_pool(name="mkpool", bufs=2) as mkpool,
            tc.tile_pool(name="vtpool", bufs=2) as vtpool,
            tc.tile_pool(name="expool", bufs=2) as expool,
            tc.tile_pool(name="spool", bufs=2) as spool,
            tc.tile_pool(name="ftpool", bufs=2) as ftpool,
            tc.tile_pool(name="rpool", bufs=2) as rpool,
            tc.tile_pool(name="mmps", bufs=2, space=bass.MemorySpace.PSUM) as mmps,
            tc.tile_pool(name="colps", bufs=1, space=bass.MemorySpace.PSUM) as colpool,
            tc.tile_pool(name="stps", bufs=1, space=bass.MemorySpace.PSUM) as stps,
        ):
            xts = {}
            xns = {}
            stvs = {}

            def load_x(bb, fine=False):
                xt = xpool.tile([128, CT, HW], F32, tag="xt")
                xts[bb] = xt
                # per-c-tile chunks so bn_stats can start before the full
                # load; the prologue batch splits finer (per bn_stats chunk)
                # to start the very first stats op earlier
                for t in range(CT):
                    if fine:
                        for sg in range(2):
                            nc.sync.dma_start(
                                out=xt[:, t, sg * 512:(sg + 1) * 512],
                                in_=x_d[bb, t * 128:(t + 1) * 128,
                                        sg * 512:(sg + 1) * 512])
                    else:
                        nc.sync.dma_start(
                            out=xt[:, t],
                            in_=x_d[bb, t * 128:(t + 1) * 128, :])
                return xt

            load_x(0, fine=True)

            # ---- HAM warmup first: its DVE dependencies lead the DVE queue
            # so the PE starts warming ~5us earlier; keeps the clock gate at
            # 8/8 (2.4GHz) until the first real matmul is ready.
            ones_st = wpool.tile([128, 2, 128], F32)
            nc.vector.memset(ones_st, 1.0)
            ones_r = wpool.tile([128, 128], MM_DT)
            nc.vector.tensor_copy(ones_r, ones_st[:, 0])
            warm_st = wpool.tile([128, 512], F32)
            nc.vector.memset(warm_st, 0.0)
            warm_rhs = wpool.tile([128, 512], MM_DT)
            nc.vector.tensor_copy(warm_rhs, warm_st)
            warm_ps = stps.tile([128, 512], F32, tag="gps")
            nc.tensor.matmul(warm_ps, lhsT=ones_r, rhs=warm_rhs,
                             start=True, stop=True)
            # consumer gated on the FIRST warm matmul: keeps the warmup from
            # being dead-code-eliminated (a consumer-less warmup measured
            # slower - eliminated) without head-of-line blocking the DVE
            # queue until warm-end (+12us on batch-0 stats)
            warm_out = wpool.tile([128, 4], F32)
            nc.vector.tensor_copy(warm_out, warm_ps[:, 0:4])
            nc.sync.dma_start(out=warmdump_d[:, :], in_=warm_out)
            for w in range(15):
                nc.tensor.matmul(warm_ps, lhsT=ones_r, rhs=warm_rhs,
                                 start=True, stop=True)

            # ---- tiny constants ----
            eps_sb = wpool.tile([128, 1], F32)
            nc.vector.memset(eps_sb, EPS)
            eshift_sb = wpool.tile([128, 1], F32)
            nc.vector.memset(eshift_sb, ESHIFT)
            ones_sb = wpool.tile([128, 2, 128], FP8)
            nc.vector.tensor_copy(ones_sb, ones_st)

            sel_st = wpool.tile([128, CT, GROUPS], F32)
            nc.sync.dma_start(out=sel_st, in_=sel_d.rearrange("(t p) g -> p t g", p=128))
            sel_sb = wpool.tile([128, CT, GROUPS], MM_DT)
            nc.vector.tensor_copy(sel_sb, sel_st)
            selT_st = wpool.tile([GROUPS, C], F32)
            nc.sync.dma_start(out=selT_st, in_=selT_d[:, :])
            selT_sb = wpool.tile([GROUPS, C], MM_DT)
            nc.vector.tensor_copy(selT_sb, selT_st)
            outb_sb = wpool.tile([128, CT], F32)
            nc.sync.dma_start(out=outb_sb, in_=outb_d.rearrange("(m p) -> p m", p=128))
            gamma_sb = wpool.tile([128, CT], F32)
            nc.sync.dma_start(out=gamma_sb, in_=gamma_d.rearrange("(m p) -> p m", p=128))
            beta_sb = wpool.tile([128, CT], F32)
            nc.sync.dma_start(out=beta_sb, in_=beta_d.rearrange("(m p) -> p m", p=128))

            # ---- folded fp8 weights ----
            wm_sb = wpool.tile([128, CT, C], FP8)
            wov_sb = wpool.tile([128, CT, C], FP8)
            wmT_r = wmT_d.rearrange("(t p) o -> p t o", p=128)
            wovT_r = wovT_d.rearrange("(t p) o -> p t o", p=128)
            for t in range(CT):
                nc.sync.dma_start(out=wm_sb[:, t], in_=wmT_r[:, t])
                nc.sync.dma_start(out=wov_sb[:, t], in_=wovT_r[:, t])

            def stats_front(bb):
                """bn stats chain for batch bb (DVE + tiny PE matmuls).
                Returns the group-stat PSUM tile; finish with stats_back."""
                xt = xts[bb]
                stats3 = spool.tile([128, CT, 4], F32, tag="stats3")
                stats3r = spool.tile([128, CT, 4], MM_DT, tag="stats3r")
                nc.vector.memset(stats3[:, :, 3:4], 0.0)
                gps = stps.tile([GROUPS, 4], F32, tag="gps")
                for t in range(CT):
                    st6 = spool.tile([128, 2, 6], F32, tag="st6")
                    for sg in range(2):
                        nc.vector.bn_stats(out=st6[:, sg], in_=xt[:, t, sg * 512:(sg + 1) * 512])
                    nc.vector.bn_aggr(out=stats3[:, t, 0:2], in_=st6)
                    nc.vector.tensor_mul(stats3[:, t, 2:3], stats3[:, t, 0:1], stats3[:, t, 0:1])
                    nc.vector.tensor_copy(stats3r[:, t], stats3[:, t])
                    nc.tensor.matmul(gps, lhsT=sel_sb[:, t], rhs=stats3r[:, t],
                                     start=(t == 0), stop=(t == CT - 1))
                return gps

            def stats_mid(bb, gps):
                """group var -> rstd (DVE small ops; sqrt emitted on ACT by
                caller-controlled ordering), then channel broadcast (PE)."""
                gsb = spool.tile([GROUPS, 4], F32, tag="gsb")
                nc.vector.tensor_copy(gsb, gps)
                gs = spool.tile([GROUPS, 4], F32, tag="gs")
                nc.vector.memset(gs, 0.0)
                tmp8 = spool.tile([GROUPS, 1], F32, tag="tmp8")
                nc.vector.tensor_mul(tmp8, gsb[:, 0:1], gsb[:, 0:1])
                nc.vector.tensor_add(gs[:, 1:2], gsb[:, 1:2], gsb[:, 2:3])
                nc.vector.tensor_sub(gs[:, 1:2], gs[:, 1:2], tmp8)
                # rstd = exp(-0.5*ln(var+eps)): stays inside ACT's resident
                # ln/exp table (a Sqrt here would force a table reload per
                # batch, ~1.5us each)
                nc.scalar.activation(gs[:, 1:2], gs[:, 1:2],
                                     mybir.ActivationFunctionType.Ln,
                                     bias=eps_sb[:GROUPS])
                nc.scalar.activation(gs[:, 1:2], gs[:, 1:2],
                                     mybir.ActivationFunctionType.Exp,
                                     scale=-0.5)
                nc.vector.tensor_copy(gs[:, 0:1], gsb[:, 0:1])
                gsr = spool.tile([GROUPS, 4], MM_DT, tag="gsr")
                nc.vector.tensor_copy(gsr, gs)
                csps = stps.tile([128, CT, 4], F32, tag="csps")
                for t in range(CT):
                    nc.tensor.matmul(csps[:, t], lhsT=selT_sb[:, t * 128:(t + 1) * 128],
                                     rhs=gsr, start=True, stop=True)
                # per-channel affine: xn = x * s + tt
                stv = spool.tile([128, CT, 2], F32, tag="stv")
                tmpc = spool.tile([128, CT, 1], F32, tag="tmpc")
                nc.vector.tensor_mul(stv[:, :, 0:1], csps[:, :, 1:2],
                                     gamma_sb.rearrange("p (t o) -> p t o", o=1))
                nc.vector.tensor_mul(tmpc, csps[:, :, 0:1], stv[:, :, 0:1])
                nc.vector.tensor_sub(stv[:, :, 1:2],
                                     beta_sb.rearrange("p (t o) -> p t o", o=1), tmpc)
                stvs[bb] = stv

            def xn_apply(bb, fast=False):
                """normalize+quantize x -> fp8 xn. Steady-state batches run
                serially on the (otherwise idle) gpsimd; the prologue batch
                fans out across three engines to cut pipeline-fill latency."""
                xt = xts[bb]
                stv = stvs.pop(bb)
                xn = xnpool.tile([128, CT, HW], FP8, tag="xn")
                xns[bb] = xn
                for t in range(CT):
                    if fast and t == 1:
                        nc.vector.tensor_scalar(
                            out=xn[:, t], in0=xt[:, t],
                            scalar1=stv[:, t, 0:1], scalar2=stv[:, t, 1:2],
                            op0=mybir.AluOpType.mult, op1=mybir.AluOpType.add)
                    elif fast and t == 3:
                        nc.scalar.activation(
                            xn[:, t], xt[:, t],
                            mybir.ActivationFunctionType.Identity,
                            bias=stv[:, t, 1:2], scale=stv[:, t, 0:1])
                    else:
                        nc.gpsimd.tensor_scalar(
                            out=xn[:, t], in0=xt[:, t],
                            scalar1=stv[:, t, 0:1], scalar2=stv[:, t, 1:2],
                            op0=mybir.AluOpType.mult, op1=mybir.AluOpType.add)
                return xn

            def proj(bb):
                """mk and vv projections (fp8 DoubleRow)."""
                xn = xns[bb]
                mk = mkpool.tile([128, CT, HW], FP8, tag="mk")
                for m in range(CT):
                    ps = mmps.tile([128, 1024], F32, tag="mm")
                    for n in range(NB):
                        for tp in range(2):
                            nc.tensor.matmul(
                                ps[:, n * 512:(n + 1) * 512],
                                lhsT=wm_sb[:, 2 * tp:2 * tp + 2, m * 128:(m + 1) * 128],
                                rhs=xn[:, 2 * tp:2 * tp + 2, n * 512:(n + 1) * 512],
                                start=(tp == 0), stop=(tp == 1), perf_mode=DR)
                    # all mk conversions on ACT: they run right after exp in
                    # the ACT queue, well before the next scores needs them.
                    # (A DVE split lands too late — behind the epilogue STTs
                    # in DVE's FIFO — stalling scores(bb+1) ~3us per batch.)
                    nc.scalar.activation(mk[:, m], ps,
                                         mybir.ActivationFunctionType.Copy)
                vT = vtpool.tile([128, PT, C], FP8, tag="vT")
                for pp in range(4):
                    ps = mmps.tile([128, 1024], F32, tag="mm")
                    for h in range(2):
                        p = 2 * pp + h
                        for tp in range(2):
                            nc.tensor.matmul(
                                ps[:, h * 512:(h + 1) * 512],
                                lhsT=xn[:, 2 * tp:2 * tp + 2, p * 128:(p + 1) * 128],
                                rhs=wov_sb[:, 2 * tp:2 * tp + 2, :],
                                start=(tp == 0), stop=(tp == 1), perf_mode=DR)
                    # undo the 8x host pre-scale of wov (gpsimd cannot read
                    # PSUM, so this conversion copy rides on ACT)
                    nc.scalar.activation(vT[:, 2 * pp:2 * pp + 2, :], ps,
                                         mybir.ActivationFunctionType.Copy,
                                         scale=1.0 / WS)
                return mk, vT

            def scores_exp(bb, mk, xn, interleave=None):
                """sT = mk^T xn then e = exp(sT*scale/8 - 2) -> fp8.
                interleave: optional list of (after_jm, fn) to slot extra
                engine work into the PE/ACT streams mid-phase."""
                expT = expool.tile([128, PT, HW], FP8, tag="expT")
                for jm in range(PT):
                    ps = mmps.tile([128, 1024], F32, tag="mm")
                    for n in range(NB):
                        for tp in range(2):
                            nc.tensor.matmul(
                                ps[:, n * 512:(n + 1) * 512],
                                lhsT=mk[:, 2 * tp:2 * tp + 2, jm * 128:(jm + 1) * 128],
                                rhs=xn[:, 2 * tp:2 * tp + 2, n * 512:(n + 1) * 512],
                                start=(tp == 0), stop=(tp == 1), perf_mode=DR)
                    nc.scalar.activation(expT[:, jm], ps,
                                         mybir.ActivationFunctionType.Exp,
                                         scale=SCALE / WS, bias=eshift_sb)
                    if interleave:
                        for after, fn in interleave:
                            if after == jm:
                                fn()
                return expT

            def colsum(bb, expT):
                colps = colpool.tile([128, HW], F32, tag="col")
                for n in range(NB):
                    for jp in range(4):
                        nc.tensor.matmul(
                            colps[:, n * 512:(n + 1) * 512],
                            lhsT=ones_sb,
                            rhs=expT[:, 2 * jp:2 * jp + 2, n * 512:(n + 1) * 512],
                            start=(jp == 0), stop=(jp == 3), perf_mode=DR)
                # 1/col as exp(-ln(col)): ACT table ops, ~3x cheaper than the
                # DVE RECIPROCAL microcode and off the DVE critical path
                lcol = rpool.tile([128, HW], F32, tag="lcol")
                nc.scalar.activation(lcol, colps,
                                     mybir.ActivationFunctionType.Ln)
                recip = rpool.tile([128, HW], F32, tag="recip")
                nc.scalar.activation(recip, lcol,
                                     mybir.ActivationFunctionType.Exp,
                                     scale=-1.0)
                return recip

            def attn_out(bb, vT, expT, recip):
                """res = vv @ e, then out = x + (res + outb)*recip, store."""
                xt = xts.pop(bb)
                for m in range(CT):
                    ps = mmps.tile([128, 1024], F32, tag="mm")
                    for n in range(NB):
                        for jp in range(4):
                            nc.tensor.matmul(
                                ps[:, n * 512:(n + 1) * 512],
                                lhsT=vT[:, 2 * jp:2 * jp + 2, m * 128:(m + 1) * 128],
                                rhs=expT[:, 2 * jp:2 * jp + 2, n * 512:(n + 1) * 512],
                                start=(jp == 0), stop=(jp == 3), perf_mode=DR)
                    ftmp = ftpool.tile([128, HW], F32, tag="ftmp")
                    nc.vector.scalar_tensor_tensor(
                        out=ftmp, in0=ps, scalar=outb_sb[:, m:m + 1], in1=recip,
                        op0=mybir.AluOpType.add, op1=mybir.AluOpType.mult)
                    # last batch: residual add on DVE (3.5x faster than the
                    # gpsimd Add) — it is the kernel's tail with nothing to
                    # overlap; other batches keep it on the idle gpsimd
                    if bb == B_PER_CORE - 1:
                        nc.vector.tensor_add(xt[:, m], ftmp, xt[:, m])
                    else:
                        nc.gpsimd.tensor_add(xt[:, m], ftmp, xt[:, m])
                    nc.sync.dma_start(
                        out=out_d[bb, m * 128:(m + 1) * 128, :],
                        in_=xt[:, m])

            # ---- software pipeline over batches ----
            # prologue: stats+apply for batch 0
            gps0 = stats_front(0)
            stats_mid(0, gps0)
            xn_apply(0, fast=True)
            for bb in range(B_PER_CORE):
                if bb + 1 < B_PER_CORE:
                    load_x(bb + 1)
                mk, vT = proj(bb)
                # next batch's stats chain is slotted into the scores phase:
                # DVE runs bn stats while PE is ACT(exp)-paced; the tiny
                # group matmuls ride between score chunks.
                pend = {}
                inter = None
                if bb + 1 < B_PER_CORE:
                    inter = [
                        (3, lambda b=bb + 1: pend.__setitem__('gps', stats_front(b))),
                        (5, lambda b=bb + 1: stats_mid(b, pend.pop('gps'))),
                        (6, lambda b=bb + 1: xn_apply(b)),
                    ]
                expT = scores_exp(bb, mk, xns.pop(bb), interleave=inter)
                recip = colsum(bb, expT)
                attn_out(bb, vT, expT, recip)
    return nc


_NC_CACHE = None


def kernel(x, norm_gamma, norm_beta, qkv_w, qkv_b, out_w, out_b):
    global _NC_CACHE
    if _NC_CACHE is None:
        _NC_CACHE = build_nc()
    nc = _NC_CACHE

    x = np.ascontiguousarray(np.asarray(x, np.float32).reshape(B_TOTAL, C, HW))
    qkv_w = np.asarray(qkv_w, np.float32)
    qkv_b = np.asarray(qkv_b, np.float32)
    out_w = np.asarray(out_w, np.float32)
    Wq, Wk, Wv = qkv_w[:C], qkv_w[C:2 * C], qkv_w[2 * C:]
    # folded weights, pre-scaled into fp8's normal range
    wmT = np.ascontiguousarray((WS * (Wq.T @ Wk)).T.astype(NP8))
    wovT = np.ascontiguousarray((WS * (out_w @ Wv)).T.astype(NP8))
    # v-bias contributes out_w @ bv to every pixel (softmax rows sum to 1)
    outb = np.ascontiguousarray(np.asarray(out_b, np.float32) + out_w @ qkv_b[2 * C:])
    # q/k biases shift scores by an i-only term (cancels in softmax) plus a
    # j-only term r_j = (Wk^T bq) . xn_j; zero for this model's inputs.
    rvec = Wk.T @ qkv_b[:C]
    assert np.allclose(rvec, 0.0) and np.allclose(qkv_b[:C], 0.0), \
        "nonzero q-bias not supported by folded kernel"
    gamma = np.ascontiguousarray(np.asarray(norm_gamma, np.float32))
    beta = np.ascontiguousarray(np.asarray(norm_beta, np.float32))
    cidx = np.arange(C)
    # each group = 64 channels; selector averages the 64 per-channel stats
    sel = np.ascontiguousarray((cidx[:, None] // (C // GROUPS) == np.arange(GROUPS)[None, :])
                               .astype(np.float32) / (C // GROUPS))
    selT = np.ascontiguousarray((np.arange(GROUPS)[:, None] == cidx[None, :] // (C // GROUPS))
                                .astype(np.float32))

    shared = {"wmT": wmT, "wovT": wovT, "outb": outb,
              "gamma": gamma, "beta": beta, "sel": sel, "selT": selT}
    in_maps = [{"x": x[c * B_PER_CORE:(c + 1) * B_PER_CORE], **shared}
               for c in range(N_CORES)]

    trace = bool(int(os.environ.get("KERNEL_TRACE", "0")))
    res = run_bass_kernel_spmd(nc, in_maps, list(range(N_CORES)), trace=trace)
    if trace and res.exec_time_ns is not None:
        print(f"HW exec time: {res.exec_time_ns} ns")
        print(f"(mean across cores: {res.mean_exec_time_ns} ns, "
              f"max core: {res.max_exec_time_core_id})")

    out = np.concatenate([res.results[c]["out"] for c in range(N_CORES)], axis=0)
    return out.reshape(B_TOTAL, C, 32, 32).astype(np.float32)
